# revision 20
# baseline (speedup 1.0000x reference)
"""Trainium2 Bass kernel for KronLinear:
    out = x @ (sum_r kron(a_r, b_r)) + bias

Sharding: 2-way over tokens x 4-way over output columns across 8 cores.
fp8 e4m3 compute with DoubleRow perf mode (K=256 per matmul, 2 MACs per
PE cell per cycle), f32 PSUM accumulation, bf16 output (converted to
f32 on the host; well within the error budget).

Host: builds W = sum_r kron(a_r,b_r) (~2 GFLOP), scales by 256 so fp8
e4m3 stays in normal range, quantizes W and x to fp8, and pre-tiles
both in the DoubleRow-interleaved layout (contraction index
kappa = ktp*256 + 2*kp + ko).  Device: 32 m-tiles x 16 ktp x 2 matmuls
(N=512 out cols each, K=256), bias added on DVE.  Host divides the
gathered output by 256.

Optimization notes (staged baseline 252.6us -> ~239-241us):
 - The matmul stream runs at the DoubleRow roofline: 216ns per N=512
   matmul = 512cyc/2.4GHz + 2.5ns NX issue.  All remaining time is
   startup (HBM supply of W + first x tiles at ~0.35 MB/us aggregate
   across the two HWDGE queues) and the end-of-kernel drain tail.
 - Startup DMAs are issued in consumption order, byte-balanced across
   the SP and Activation HWDGE queues (per-queue completion is in
   issue order); mt0/mt1 run interleaved (4 MMs per ktp) to match the
   supply rate.
 - PE warm-up: ~4.8us of dummy DoubleRow matmuls on a memset scratch
   tile release the HAM clock throttle (cold 1.2GHz -> warm 2.4GHz
   needs ~3.4us of *contiguous* PE-busy) before the real stream.
 - Output DMAs issue on both HWDGE engines; the last m-tile runs three
   accumulation groups (512/384/128 cols) so earlier groups drain under
   later groups' matmuls and only a 128-col chunk follows the final
   matmul.
"""
import numpy as np

RANK = 64
A1 = A2 = B1 = B2 = 64
NTOK = 8192
NCORES = 8
TH = 2            # token shards
CQ = 4            # column shards
TOK_SH = NTOK // TH          # 4096 tokens per core
COLS_SH = (A2 * B2) // CQ    # 1024 out cols per core
JPC = A2 // CQ               # 16 j-values per core
MT = TOK_SH // 128           # 32 m-tiles
KTP = (A1 * B1) // 256       # 16 k-tile-pairs (K=256 each)
WSCALE = 256.0
NWARM = 36

_CACHE = {}


def _build_nc(debug=False):
    import sys
    if "/opt/trn_rl_repo" not in sys.path:
        sys.path.insert(0, "/opt/trn_rl_repo")
    import concourse.tile as tile
    from concourse import bacc, mybir

    f32 = mybir.dt.float32
    fp8 = mybir.dt.float8e4
    bf16 = mybir.dt.bfloat16
    DR = mybir.MatmulPerfMode.DoubleRow

    nc = bacc.Bacc(None, target_bir_lowering=False, debug=debug,
                   num_devices=NCORES, enable_partition_id=False)

    # xt[mt, kp, ktp*256 + ko*128 + m] = x[mt*128+m, ktp*256 + 2*kp + ko]
    xt_d = nc.dram_tensor("xt", [MT, 128, KTP * 256], fp8, kind="ExternalInput")
    # wsl[ktp, kp, j*128 + ko*64 + l] = 256*W[ktp*256+2*kp+ko, j*64+l]
    w_d = nc.dram_tensor("wsl", [KTP, 128, 2 * COLS_SH], fp8,
                         kind="ExternalInput")
    bias_d = nc.dram_tensor("bias", [1, COLS_SH], f32, kind="ExternalInput")
    out_d = nc.dram_tensor("out", [TOK_SH, COLS_SH], bf16,
                           kind="ExternalOutput")

    with tile.TileContext(nc) as tc:
        with tc.tile_pool(name="const", bufs=1) as cpool, \
             tc.tile_pool(name="wres", bufs=1) as wpool, \
             tc.tile_pool(name="xin", bufs=4) as xpool, \
             tc.tile_pool(name="oout", bufs=2) as opool, \
             tc.tile_pool(name="mps", bufs=3, space="PSUM") as mps_pool, \
             tc.tile_pool(name="lps", bufs=1, space="PSUM") as lps_pool:
            # PSUM: mps 3x[128,1024] = 6 banks, lps 2 tail tiles = 2 banks

            w_sb = []
            for ktp in range(KTP):
                wt = wpool.tile([128, 2 * COLS_SH], fp8, tag=f"w{ktp}")
                w_sb.append(wt)
            bias_sb = cpool.tile([128, COLS_SH], f32)
            scratch = cpool.tile([128, 512], fp8, tag="warm")

            xts = [xpool.tile([128, KTP * 256], fp8, tag="xts",
                              name=f"xts{i}") for i in range(4)]

            # ---- startup DMA issues, in consumption order, byte-balanced
            # across the two HWDGE queues.
            H = KTP * 128  # half an x tile, contiguous 2KB rows
            nc.scalar.dma_start(out=w_sb[0][:], in_=w_d[0, :, :])
            nc.sync.dma_start(out=xts[0][:, 0:H], in_=xt_d[0, :, 0:H])
            nc.sync.dma_start(out=w_sb[1][:], in_=w_d[1, :, :])
            nc.scalar.dma_start(out=xts[1][:, 0:H], in_=xt_d[1, :, 0:H])
            nc.sync.dma_start(out=w_sb[3][:], in_=w_d[3, :, :])
            nc.scalar.dma_start(out=w_sb[2][:], in_=w_d[2, :, :])
            nc.scalar.dma_start(out=w_sb[4][:], in_=w_d[4, :, :])
            nc.sync.dma_start(out=w_sb[5][:], in_=w_d[5, :, :])
            nc.scalar.dma_start(out=w_sb[6][:], in_=w_d[6, :, :])
            nc.sync.dma_start(out=xts[0][:, H:], in_=xt_d[0, :, H:])
            nc.scalar.dma_start(out=xts[1][:, H:], in_=xt_d[1, :, H:])
            nc.scalar.dma_start(out=xts[2][:, H:], in_=xt_d[2, :, H:])
            for k in range(7, KTP):
                eng = nc.scalar if (k % 2 == 0) else nc.sync
                eng.dma_start(out=w_sb[k][:], in_=w_d[k, :, :])
            nc.sync.dma_start(out=xts[2][:, 0:H], in_=xt_d[2, :, 0:H])
            nc.scalar.dma_start(out=xts[3][:], in_=xt_d[3, :, :])
            nc.scalar.dma_start(
                out=bias_sb[:],
                in_=bias_d[:, :].broadcast_to([128, COLS_SH]))

            # ---- PE warm-up (HAM clock-gate release): ~4.8us of
            # contiguous dummy N=128 DR matmuls.
            phB = lps_pool.tile([128, 128], f32, tag="lhB")
            ps_warm = mps_pool.tile([128, COLS_SH], f32, tag="ps")
            nc.gpsimd.memset(scratch[:], 0)
            wl = scratch[:, 0:256].rearrange("p (ko m) -> p ko m", ko=2)
            for _ in range(NWARM):
                nc.tensor.matmul(ps_warm[:, 0:128], wl, wl,
                                 start=True, stop=True, perf_mode=DR)

            # ---- matmul + drain helpers.
            def mm(ps_ap, xt, ktp, h, first, last):
                lt = xt[:, ktp * 256:(ktp + 1) * 256] \
                    .rearrange("p (ko m) -> p ko m", ko=2)
                wv = w_sb[ktp][:, :] \
                    .rearrange("p (j ko l) -> p j ko l", ko=2, l=64)
                rhs = wv[:, 8 * h:8 * h + 8, :, :] \
                    .transpose([0, 2, 1, 3])
                nc.tensor.matmul(ps_ap, lt, rhs,
                                 start=first, stop=last, perf_mode=DR)

            def drain(mt, ps, cols, chunks, engines, osb=None, ps_off=0):
                if osb is None:
                    osb = opool.tile([128, COLS_SH], bf16, tag="osb")
                lo, hi = cols
                cw = (hi - lo) // chunks
                for c in range(chunks):
                    sl = slice(lo + cw * c, lo + cw * c + cw)
                    psl = slice(sl.start - ps_off, sl.stop - ps_off)
                    nc.vector.tensor_add(osb[:, sl], ps[:, psl],
                                         bias_sb[:, sl])
                    engines[c % len(engines)].dma_start(
                        out=out_d[mt * 128:(mt + 1) * 128, sl],
                        in_=osb[:, sl])
                return osb

            # mt 0 and 1 interleaved (4 MMs per ktp) to match the HBM
            # supply rate while W trickles in.
            ps0 = mps_pool.tile([128, COLS_SH], f32, tag="ps")
            ps1 = mps_pool.tile([128, COLS_SH], f32, tag="ps")
            ps2 = mps_pool.tile([128, COLS_SH], f32, tag="ps")
            # mt2 joins at ktp 9 (K-sum order is free): consumption of the
            # late W tiles runs at 6 MMs/tile instead of 4, absorbing
            # per-run HBM service jitter; mt2's remaining ktps use
            # SBUF-resident W afterwards with zero supply risk.
            JOIN = 9
            for ktp in range(KTP):
                first, last = ktp == 0, ktp == KTP - 1
                mm(ps0[:, 0:512], xts[0], ktp, 0, first, last)
                mm(ps0[:, 512:1024], xts[0], ktp, 1, first, last)
                mm(ps1[:, 0:512], xts[1], ktp, 0, first, last)
                mm(ps1[:, 512:1024], xts[1], ktp, 1, first, last)
                if ktp >= JOIN:
                    f2 = ktp == JOIN
                    mm(ps2[:, 0:512], xts[2], ktp, 0, f2, False)
                    mm(ps2[:, 512:1024], xts[2], ktp, 1, f2, False)
            drain(0, ps0, (0, COLS_SH), 2, [nc.scalar, nc.sync])
            drain(1, ps1, (0, COLS_SH), 2, [nc.scalar, nc.sync])
            for ktp in range(JOIN):
                l2 = ktp == JOIN - 1
                mm(ps2[:, 0:512], xts[2], ktp, 0, False, l2)
                mm(ps2[:, 512:1024], xts[2], ktp, 1, False, l2)
            drain(2, ps2, (0, COLS_SH), 2, [nc.scalar, nc.sync])

            for mt in range(3, MT - 1):
                if mt >= 4:
                    xts.append(xpool.tile([128, KTP * 256], fp8, tag="xts",
                                          name=f"xts{mt}"))
                    nc.sync.dma_start(out=xts[mt][:], in_=xt_d[mt, :, :])
                ps = mps_pool.tile([128, COLS_SH], f32, tag="ps")
                for ktp in range(KTP):
                    first, last = ktp == 0, ktp == KTP - 1
                    mm(ps[:, 0:512], xts[mt], ktp, 0, first, last)
                    mm(ps[:, 512:1024], xts[mt], ktp, 1, first, last)
                drain(mt, ps, (0, COLS_SH), 2, [nc.scalar, nc.sync])

            # Last m-tile in three accumulation groups (512/384/128 cols).
            mt = MT - 1
            xts.append(xpool.tile([128, KTP * 256], fp8, tag="xts",
                                  name=f"xts{mt}"))
            nc.sync.dma_start(out=xts[mt][:], in_=xt_d[mt, :, :])
            ps = mps_pool.tile([128, COLS_SH], f32, tag="ps")
            phA = lps_pool.tile([128, 384], f32, tag="lhA")
            for ktp in range(KTP):
                mm(ps[:, 0:512], xts[mt], ktp, 0, ktp == 0, ktp == KTP - 1)
            osb = drain(mt, ps, (0, 512), 2, [nc.scalar, nc.sync])

            def mmj(ps_ap, ktp, jlo, jhi, first, last):
                lt = xts[mt][:, ktp * 256:(ktp + 1) * 256] \
                    .rearrange("p (ko m) -> p ko m", ko=2)
                wv = w_sb[ktp][:, :] \
                    .rearrange("p (j ko l) -> p j ko l", ko=2, l=64)
                rhs = wv[:, jlo:jhi, :, :].transpose([0, 2, 1, 3])
                nc.tensor.matmul(ps_ap, lt, rhs,
                                 start=first, stop=last, perf_mode=DR)

            for ktp in range(KTP):
                mmj(phA[:, :], ktp, 8, 14, ktp == 0, ktp == KTP - 1)
            nc.vector.tensor_add(osb[:, 512:704], phA[:, 0:192],
                                 bias_sb[:, 512:704])
            nc.scalar.dma_start(out=out_d[mt * 128:(mt + 1) * 128, 512:704],
                                in_=osb[:, 512:704])
            nc.vector.tensor_add(osb[:, 704:896], phA[:, 192:384],
                                 bias_sb[:, 704:896])
            nc.sync.dma_start(out=out_d[mt * 128:(mt + 1) * 128, 704:896],
                              in_=osb[:, 704:896])
            for ktp in range(KTP):
                mmj(phB[:, :], ktp, 14, 16, ktp == 0, ktp == KTP - 1)
            nc.vector.tensor_add(osb[:, 896:1024], phB[:, :],
                                 bias_sb[:, 896:1024])
            # final chunk row-split across both HWDGE queues: 64 packets
            # each in parallel instead of 128 serial on one queue.
            nc.scalar.dma_start(
                out=out_d[mt * 128:mt * 128 + 64, 896:1024],
                in_=osb[0:64, 896:1024])
            nc.sync.dma_start(
                out=out_d[mt * 128 + 64:(mt + 1) * 128, 896:1024],
                in_=osb[64:128, 896:1024])

    nc.compile()
    return nc


def _host_prep(x, a, b, bias):
    """Build per-core input maps. W built host-side, fp8 DR layouts."""
    import ml_dtypes
    f8 = ml_dtypes.float8_e4m3fn
    x = np.asarray(x, dtype=np.float32)
    a = np.asarray(a, dtype=np.float32)
    b = np.asarray(b, dtype=np.float32)
    bias = np.asarray(bias, dtype=np.float32)

    # W[(i,k),(j,l)] = sum_r a[r,i,j] b[r,k,l], scaled by WSCALE
    amat = a.transpose(1, 2, 0).reshape(A1 * A2, RANK)      # [(i,j), r]
    bmat = b.reshape(RANK, B1 * B2)                         # [r, (k,l)]
    wtmp = (amat @ bmat).reshape(A1, A2, B1, B2)            # [i, j, k, l]
    w = np.ascontiguousarray(wtmp.transpose(0, 2, 1, 3))    # [i, k, j, l]
    w = (w.reshape(A1 * B1, A2 * B2) * WSCALE).astype(f8)

    xt_by_th = []
    for th in range(TH):
        xh = x[th * TOK_SH:(th + 1) * TOK_SH]
        # xt[mt, kp, ktp, ko, m] = x[mt*128+m, ktp*256 + 2*kp + ko]
        x5 = xh.reshape(MT, 128, KTP, 128, 2)               # [mt, m, ktp, kp, ko]
        xt = np.ascontiguousarray(
            x5.transpose(0, 3, 2, 4, 1)).reshape(MT, 128, KTP * 256).astype(f8)
        xt_by_th.append(xt)
    w_by_cq = []
    bias_by_cq = []
    for cq in range(CQ):
        wsl = w[:, cq * COLS_SH:(cq + 1) * COLS_SH]         # [4096, 1024]
        # [ktp, kp, ko, j, l] -> [ktp, kp, j, ko, l]
        w5 = wsl.reshape(KTP, 128, 2, JPC, 64)
        w_by_cq.append(np.ascontiguousarray(
            w5.transpose(0, 1, 3, 2, 4)).reshape(KTP, 128, 2 * COLS_SH))
        bias_by_cq.append(np.ascontiguousarray(
            (bias[cq * COLS_SH:(cq + 1) * COLS_SH] * WSCALE)
            .reshape(1, COLS_SH)))

    in_maps = []
    for c in range(NCORES):
        th, cq = c // CQ, c % CQ
        in_maps.append({
            "xt": xt_by_th[th],
            "wsl": w_by_cq[cq],
            "bias": bias_by_cq[cq],
        })
    return in_maps


def kernel(x, a, b, bias):
    import sys
    if "/opt/trn_rl_repo" not in sys.path:
        sys.path.insert(0, "/opt/trn_rl_repo")
    from concourse.bass_utils import run_bass_kernel_spmd

    if "nc" not in _CACHE:
        _CACHE["nc"] = _build_nc(debug=False)
    nc = _CACHE["nc"]

    in_maps = _host_prep(x, a, b, bias)
    res = run_bass_kernel_spmd(nc, in_maps, core_ids=list(range(NCORES)))
    out = np.empty((NTOK, A2 * B2), dtype=np.float32)
    inv = np.float32(1.0 / WSCALE)
    for c in range(NCORES):
        th, cq = c // CQ, c % CQ
        np.multiply(res.results[c]["out"].astype(np.float32), inv,
                    out=out[th * TOK_SH:(th + 1) * TOK_SH,
                            cq * COLS_SH:(cq + 1) * COLS_SH])
    return out


# revision 21
# speedup vs baseline: 1.0049x; 1.0049x over previous
"""Trainium2 Bass kernel for KronLinear:
    out = x @ (sum_r kron(a_r, b_r)) + bias

Sharding: 2-way over tokens x 4-way over output columns across 8 cores.
fp8 e4m3 compute with DoubleRow perf mode (K=256 per matmul, 2 MACs per
PE cell per cycle), f32 PSUM accumulation, bf16 output (converted to
f32 on the host; well within the error budget).

Host: builds W = sum_r kron(a_r,b_r) (~2 GFLOP), scales by 256 so fp8
e4m3 stays in normal range, quantizes W and x to fp8, and pre-tiles
both in the DoubleRow-interleaved layout (contraction index
kappa = ktp*256 + 2*kp + ko).  Device: 32 m-tiles x 16 ktp x 2 matmuls
(N=512 out cols each, K=256), bias added on DVE.  Host divides the
gathered output by 256.

Optimization notes (staged baseline 252.6us -> ~239-241us):
 - The matmul stream runs at the DoubleRow roofline: 216ns per N=512
   matmul = 512cyc/2.4GHz + 2.5ns NX issue.  All remaining time is
   startup (HBM supply of W + first x tiles at ~0.35 MB/us aggregate
   across the two HWDGE queues) and the end-of-kernel drain tail.
 - Startup DMAs are issued in consumption order, byte-balanced across
   the SP and Activation HWDGE queues (per-queue completion is in
   issue order); mt0/mt1 run interleaved (4 MMs per ktp) to match the
   supply rate.
 - PE warm-up: ~4.8us of dummy DoubleRow matmuls on a memset scratch
   tile release the HAM clock throttle (cold 1.2GHz -> warm 2.4GHz
   needs ~3.4us of *contiguous* PE-busy) before the real stream.
 - Output DMAs issue on both HWDGE engines; the last m-tile runs three
   accumulation groups (512/384/128 cols) so earlier groups drain under
   later groups' matmuls and only a 128-col chunk follows the final
   matmul.
"""
import numpy as np

RANK = 64
A1 = A2 = B1 = B2 = 64
NTOK = 8192
NCORES = 8
TH = 2            # token shards
CQ = 4            # column shards
TOK_SH = NTOK // TH          # 4096 tokens per core
COLS_SH = (A2 * B2) // CQ    # 1024 out cols per core
JPC = A2 // CQ               # 16 j-values per core
MT = TOK_SH // 128           # 32 m-tiles
KTP = (A1 * B1) // 256       # 16 k-tile-pairs (K=256 each)
WSCALE = 256.0
NWARM = 36

_CACHE = {}


def _build_nc(debug=False):
    import sys
    if "/opt/trn_rl_repo" not in sys.path:
        sys.path.insert(0, "/opt/trn_rl_repo")
    import concourse.tile as tile
    from concourse import bacc, mybir

    f32 = mybir.dt.float32
    fp8 = mybir.dt.float8e4
    bf16 = mybir.dt.bfloat16
    DR = mybir.MatmulPerfMode.DoubleRow

    nc = bacc.Bacc(None, target_bir_lowering=False, debug=debug,
                   num_devices=NCORES, enable_partition_id=False)

    # xt[mt, kp, ktp*256 + ko*128 + m] = x[mt*128+m, ktp*256 + 2*kp + ko]
    xt_d = nc.dram_tensor("xt", [MT, 128, KTP * 256], fp8, kind="ExternalInput")
    # wsl[ktp, kp, j*128 + ko*64 + l] = 256*W[ktp*256+2*kp+ko, j*64+l]
    w_d = nc.dram_tensor("wsl", [KTP, 128, 2 * COLS_SH], fp8,
                         kind="ExternalInput")
    bias_d = nc.dram_tensor("bias", [1, COLS_SH], f32, kind="ExternalInput")
    out_d = nc.dram_tensor("out", [TOK_SH, COLS_SH], bf16,
                           kind="ExternalOutput")

    with tile.TileContext(nc) as tc:
        with tc.tile_pool(name="const", bufs=1) as cpool, \
             tc.tile_pool(name="wres", bufs=1) as wpool, \
             tc.tile_pool(name="xin", bufs=4) as xpool, \
             tc.tile_pool(name="oout", bufs=2) as opool, \
             tc.tile_pool(name="mps", bufs=3, space="PSUM") as mps_pool, \
             tc.tile_pool(name="lps", bufs=1, space="PSUM") as lps_pool:
            # PSUM: mps 3x[128,1024] = 6 banks, lps 2 tail tiles = 2 banks

            w_sb = []
            for ktp in range(KTP):
                wt = wpool.tile([128, 2 * COLS_SH], fp8, tag=f"w{ktp}")
                w_sb.append(wt)
            bias_sb = cpool.tile([128, COLS_SH], f32)
            scratch = cpool.tile([128, 512], fp8, tag="warm")

            xts = [xpool.tile([128, KTP * 256], fp8, tag="xts",
                              name=f"xts{i}") for i in range(4)]

            # ---- startup DMA issues, in consumption order, byte-balanced
            # across the two HWDGE queues.
            H = KTP * 128  # half an x tile, contiguous 2KB rows
            nc.scalar.dma_start(out=w_sb[0][:], in_=w_d[0, :, :])
            nc.sync.dma_start(out=xts[0][:, 0:H], in_=xt_d[0, :, 0:H])
            nc.sync.dma_start(out=w_sb[1][:], in_=w_d[1, :, :])
            nc.scalar.dma_start(out=xts[1][:, 0:H], in_=xt_d[1, :, 0:H])
            nc.sync.dma_start(out=w_sb[3][:], in_=w_d[3, :, :])
            nc.scalar.dma_start(out=w_sb[2][:], in_=w_d[2, :, :])
            nc.scalar.dma_start(out=w_sb[4][:], in_=w_d[4, :, :])
            nc.sync.dma_start(out=w_sb[5][:], in_=w_d[5, :, :])
            nc.scalar.dma_start(out=w_sb[6][:], in_=w_d[6, :, :])
            nc.sync.dma_start(out=xts[0][:, H:], in_=xt_d[0, :, H:])
            nc.scalar.dma_start(out=xts[1][:, H:], in_=xt_d[1, :, H:])
            for k in range(7, KTP):
                eng = nc.scalar if (k % 2 == 0) else nc.sync
                eng.dma_start(out=w_sb[k][:], in_=w_d[k, :, :])
            nc.sync.dma_start(out=xts[2][:, 0:H], in_=xt_d[2, :, 0:H])
            nc.sync.dma_start(out=xts[2][:, H:], in_=xt_d[2, :, H:])
            nc.scalar.dma_start(out=xts[3][:], in_=xt_d[3, :, :])
            nc.scalar.dma_start(
                out=bias_sb[:],
                in_=bias_d[:, :].broadcast_to([128, COLS_SH]))

            # ---- PE warm-up (HAM clock-gate release): ~4.8us of
            # contiguous dummy N=128 DR matmuls.
            phB = lps_pool.tile([128, 128], f32, tag="lhB")
            ps_warm = mps_pool.tile([128, COLS_SH], f32, tag="ps")
            nc.gpsimd.memset(scratch[:], 0)
            wl = scratch[:, 0:256].rearrange("p (ko m) -> p ko m", ko=2)
            for _ in range(NWARM):
                nc.tensor.matmul(ps_warm[:, 0:128], wl, wl,
                                 start=True, stop=True, perf_mode=DR)

            # ---- matmul + drain helpers.
            def mm(ps_ap, xt, ktp, h, first, last):
                lt = xt[:, ktp * 256:(ktp + 1) * 256] \
                    .rearrange("p (ko m) -> p ko m", ko=2)
                wv = w_sb[ktp][:, :] \
                    .rearrange("p (j ko l) -> p j ko l", ko=2, l=64)
                rhs = wv[:, 8 * h:8 * h + 8, :, :] \
                    .transpose([0, 2, 1, 3])
                nc.tensor.matmul(ps_ap, lt, rhs,
                                 start=first, stop=last, perf_mode=DR)

            def drain(mt, ps, cols, chunks, engines, osb=None, ps_off=0):
                if osb is None:
                    osb = opool.tile([128, COLS_SH], bf16, tag="osb")
                lo, hi = cols
                cw = (hi - lo) // chunks
                for c in range(chunks):
                    sl = slice(lo + cw * c, lo + cw * c + cw)
                    psl = slice(sl.start - ps_off, sl.stop - ps_off)
                    nc.vector.tensor_add(osb[:, sl], ps[:, psl],
                                         bias_sb[:, sl])
                    engines[c % len(engines)].dma_start(
                        out=out_d[mt * 128:(mt + 1) * 128, sl],
                        in_=osb[:, sl])
                return osb

            # mt 0 and 1 interleaved (4 MMs per ktp) to match the HBM
            # supply rate while W trickles in.
            ps0 = mps_pool.tile([128, COLS_SH], f32, tag="ps")
            ps1 = mps_pool.tile([128, COLS_SH], f32, tag="ps")
            for ktp in range(KTP):
                first, last = ktp == 0, ktp == KTP - 1
                mm(ps0[:, 0:512], xts[0], ktp, 0, first, last)
                mm(ps0[:, 512:1024], xts[0], ktp, 1, first, last)
                mm(ps1[:, 0:512], xts[1], ktp, 0, first, last)
                mm(ps1[:, 512:1024], xts[1], ktp, 1, first, last)
            drain(0, ps0, (0, COLS_SH), 2, [nc.scalar, nc.sync])
            drain(1, ps1, (0, COLS_SH), 2, [nc.scalar, nc.sync])

            for mt in range(2, MT - 1):
                if mt >= 4:
                    xts.append(xpool.tile([128, KTP * 256], fp8, tag="xts",
                                          name=f"xts{mt}"))
                    nc.sync.dma_start(out=xts[mt][:], in_=xt_d[mt, :, :])
                ps = mps_pool.tile([128, COLS_SH], f32, tag="ps")
                for ktp in range(KTP):
                    first, last = ktp == 0, ktp == KTP - 1
                    mm(ps[:, 0:512], xts[mt], ktp, 0, first, last)
                    mm(ps[:, 512:1024], xts[mt], ktp, 1, first, last)
                drain(mt, ps, (0, COLS_SH), 2, [nc.scalar, nc.sync])

            # Last m-tile in three accumulation groups (512/384/128 cols).
            mt = MT - 1
            xts.append(xpool.tile([128, KTP * 256], fp8, tag="xts",
                                  name=f"xts{mt}"))
            nc.sync.dma_start(out=xts[mt][:], in_=xt_d[mt, :, :])
            ps = mps_pool.tile([128, COLS_SH], f32, tag="ps")
            phA = lps_pool.tile([128, 384], f32, tag="lhA")
            for ktp in range(KTP):
                mm(ps[:, 0:512], xts[mt], ktp, 0, ktp == 0, ktp == KTP - 1)
            osb = drain(mt, ps, (0, 512), 2, [nc.scalar, nc.sync])

            def mmj(ps_ap, ktp, jlo, jhi, first, last):
                lt = xts[mt][:, ktp * 256:(ktp + 1) * 256] \
                    .rearrange("p (ko m) -> p ko m", ko=2)
                wv = w_sb[ktp][:, :] \
                    .rearrange("p (j ko l) -> p j ko l", ko=2, l=64)
                rhs = wv[:, jlo:jhi, :, :].transpose([0, 2, 1, 3])
                nc.tensor.matmul(ps_ap, lt, rhs,
                                 start=first, stop=last, perf_mode=DR)

            for ktp in range(KTP):
                mmj(phA[:, :], ktp, 8, 14, ktp == 0, ktp == KTP - 1)
            nc.vector.tensor_add(osb[:, 512:704], phA[:, 0:192],
                                 bias_sb[:, 512:704])
            nc.scalar.dma_start(out=out_d[mt * 128:(mt + 1) * 128, 512:704],
                                in_=osb[:, 512:704])
            nc.vector.tensor_add(osb[:, 704:896], phA[:, 192:384],
                                 bias_sb[:, 704:896])
            nc.sync.dma_start(out=out_d[mt * 128:(mt + 1) * 128, 704:896],
                              in_=osb[:, 704:896])
            for ktp in range(KTP):
                mmj(phB[:, :], ktp, 14, 16, ktp == 0, ktp == KTP - 1)
            nc.vector.tensor_add(osb[:, 896:1024], phB[:, :],
                                 bias_sb[:, 896:1024])
            # final chunk row-split across both HWDGE queues: 64 packets
            # each in parallel instead of 128 serial on one queue.
            nc.scalar.dma_start(
                out=out_d[mt * 128:mt * 128 + 64, 896:1024],
                in_=osb[0:64, 896:1024])
            nc.sync.dma_start(
                out=out_d[mt * 128 + 64:(mt + 1) * 128, 896:1024],
                in_=osb[64:128, 896:1024])

    nc.compile()
    return nc


def _host_prep(x, a, b, bias):
    """Build per-core input maps. W built host-side, fp8 DR layouts."""
    import ml_dtypes
    f8 = ml_dtypes.float8_e4m3fn
    x = np.asarray(x, dtype=np.float32)
    a = np.asarray(a, dtype=np.float32)
    b = np.asarray(b, dtype=np.float32)
    bias = np.asarray(bias, dtype=np.float32)

    # W[(i,k),(j,l)] = sum_r a[r,i,j] b[r,k,l], scaled by WSCALE
    amat = a.transpose(1, 2, 0).reshape(A1 * A2, RANK)      # [(i,j), r]
    bmat = b.reshape(RANK, B1 * B2)                         # [r, (k,l)]
    wtmp = (amat @ bmat).reshape(A1, A2, B1, B2)            # [i, j, k, l]
    w = np.ascontiguousarray(wtmp.transpose(0, 2, 1, 3))    # [i, k, j, l]
    w = (w.reshape(A1 * B1, A2 * B2) * WSCALE).astype(f8)

    xt_by_th = []
    for th in range(TH):
        xh = x[th * TOK_SH:(th + 1) * TOK_SH]
        # xt[mt, kp, ktp, ko, m] = x[mt*128+m, ktp*256 + 2*kp + ko]
        x5 = xh.reshape(MT, 128, KTP, 128, 2)               # [mt, m, ktp, kp, ko]
        xt = np.ascontiguousarray(
            x5.transpose(0, 3, 2, 4, 1)).reshape(MT, 128, KTP * 256).astype(f8)
        xt_by_th.append(xt)
    w_by_cq = []
    bias_by_cq = []
    for cq in range(CQ):
        wsl = w[:, cq * COLS_SH:(cq + 1) * COLS_SH]         # [4096, 1024]
        # [ktp, kp, ko, j, l] -> [ktp, kp, j, ko, l]
        w5 = wsl.reshape(KTP, 128, 2, JPC, 64)
        w_by_cq.append(np.ascontiguousarray(
            w5.transpose(0, 1, 3, 2, 4)).reshape(KTP, 128, 2 * COLS_SH))
        bias_by_cq.append(np.ascontiguousarray(
            (bias[cq * COLS_SH:(cq + 1) * COLS_SH] * WSCALE)
            .reshape(1, COLS_SH)))

    in_maps = []
    for c in range(NCORES):
        th, cq = c // CQ, c % CQ
        in_maps.append({
            "xt": xt_by_th[th],
            "wsl": w_by_cq[cq],
            "bias": bias_by_cq[cq],
        })
    return in_maps


def kernel(x, a, b, bias):
    import sys
    if "/opt/trn_rl_repo" not in sys.path:
        sys.path.insert(0, "/opt/trn_rl_repo")
    from concourse.bass_utils import run_bass_kernel_spmd

    if "nc" not in _CACHE:
        _CACHE["nc"] = _build_nc(debug=False)
    nc = _CACHE["nc"]

    in_maps = _host_prep(x, a, b, bias)
    res = run_bass_kernel_spmd(nc, in_maps, core_ids=list(range(NCORES)))
    out = np.empty((NTOK, A2 * B2), dtype=np.float32)
    inv = np.float32(1.0 / WSCALE)
    for c in range(NCORES):
        th, cq = c // CQ, c % CQ
        np.multiply(res.results[c]["out"].astype(np.float32), inv,
                    out=out[th * TOK_SH:(th + 1) * TOK_SH,
                            cq * COLS_SH:(cq + 1) * COLS_SH])
    return out
